# revision 35
# baseline (speedup 1.0000x reference)
"""TRN2 Bass kernel for GPT-2 style causal self-attention (B=4, S=2048, D=1024, H=16).

Sharding: 8 cores = 4 batches x 2 head-groups (8 heads each).
Each core computes qkv projections for its (batch, head-group), runs causal
attention for its 8 heads, computes a partial c_proj, then a pairwise
ReduceScatter (replica groups [[0,1],[2,3],[4,5],[6,7]]) sums the two
head-group partials and splits the token rows between the pair.

qkv/c_proj matmuls run in float32r (full-rate at 512-wide moving operand);
the attention inner loop (scores, AV) runs in bf16 — 64-contraction and
65-partition-out f32r matmuls measure ~1.5-2x slower than bf16 on HW.
Softmax needs no max-subtraction (scores bounded ~|2.7| at this scale);
masked entries are zeroed after exp via affine_select on a 128-wide window
of the diagonal tile; the softmax denominator rides along as a 65th ones
column of V in the same AV matmul. Scores/exp/AV are causally trimmed to
the valid q-range per k-tile. c_proj for a chunk runs in two half
accumulations (heads 0-3 into SBUF after hp1, heads 4-7 added after hp3)
so the last chunk's c_proj + ReduceScatter trigger right behind the last
AV. External output is bf16 (host upcasts during gather).
"""
import sys
sys.path.insert(0, "/opt/trn_rl_repo")
import numpy as np

B, S, D, H, HD = 4, 2048, 1024, 16, 64
NCORES = 8
HPC = H // 2          # 8 heads per core
ACH = HPC * HD        # 512 local a-channels
P = 128
QCN = 4               # token chunks
QCS = S // QCN        # 512
FKT = D // P          # 8 feature k-tiles
VW = HPC * P          # 1024: per-head [64 v-dims | ones | 63 zeros]
SKEW = 2              # attention pipeline skew (score tiles ahead of AV)

_CACHE = {}


def _build():
    from concourse import bacc, tile, mybir
    from concourse.tile import add_dep_helper
    f32 = mybir.dt.float32
    f32r = mybir.dt.float32r
    bf16 = mybir.dt.bfloat16
    Exp = mybir.ActivationFunctionType.Exp

    nc = bacc.Bacc("TRN2", target_bir_lowering=False, debug=False,
                   num_devices=NCORES)
    xt_e = nc.dram_tensor("xt", [D, S], f32, kind="ExternalInput")
    wq_e = nc.dram_tensor("wq", [D, ACH], f32, kind="ExternalInput")
    wk_e = nc.dram_tensor("wk", [D, ACH], f32, kind="ExternalInput")
    wv_e = nc.dram_tensor("wv", [D, ACH], f32, kind="ExternalInput")
    wp_e = nc.dram_tensor("wp", [ACH, D], f32, kind="ExternalInput")
    out_e = nc.dram_tensor("outp", [S // 2, D], bf16,
                           kind="ExternalOutput")
    rg = [[0, 1], [2, 3], [4, 5], [6, 7]]

    with tile.TileContext(nc) as tc:
        with tc.tile_pool(name="sb", bufs=1) as sb, \
             tc.tile_pool(name="pp", bufs=1, space="PSUM") as pp, \
             tc.tile_pool(name="dr", bufs=1, space="DRAM") as dr:

            kT = [sb.tile([P, S], bf16, name=f"kTr{i}", tag="kT", bufs=4)
                  for i in range(4)]
            vx = [sb.tile([P, VW], bf16, name=f"vxr{i}", tag="vx", bufs=16)
                  for i in range(16)]
            wv_t = [sb.tile([P, ACH], f32r, name=f"wvr{i}", tag="wv", bufs=8)
                    for i in range(FKT)]
            wp_t = {(a, o): sb.tile([P, 512], f32r, name=f"wpr{a}_{o}",
                                    tag="wp", bufs=8)
                    for a in range(4) for o in range(2)}
            parts = [dr.tile([QCS, D], bf16, name=f"part{q}",
                             tag=f"pq{q}") for q in range(QCN)]
            rsos = [dr.tile([QCS // 2, D], bf16, name=f"rso{q}",
                            tag=f"rq{q}") for q in range(QCN)]

            qt_all = {}    # (qc, ct) -> tile
            at_all = {}    # (qc, j) -> tile
            oacc_all = {}  # (qc, tt, oc) -> sbuf f32 accumulator
            rs_insts = {}

            def qkv_units(qc):
                """Generator of emission closures for the qkv phase of qc."""
                xc = [sb.tile([P, QCS], f32r, name=f"xc{qc}_{k}", tag="xc",
                              bufs=10) for k in range(FKT)]

                def load_x():
                    for k in range(FKT):
                        nc.sync.dma_start(
                            out=xc[k],
                            in_=xt_e.ap()[k * P:(k + 1) * P,
                                          qc * QCS:(qc + 1) * QCS]
                                .bitcast(f32r))
                load_x.is_load = True
                yield load_x
                for proj, w_e in (("q", wq_e), ("k", wk_e)):
                    for ct in range(4):
                        w_c = sb.tile([P, FKT, P], f32r,
                                      name=f"w{proj}c{qc}_{ct}", tag="wcol",
                                      bufs=5)

                        def load_w(w_c=w_c, w_e=w_e, ct=ct):
                            nc.scalar.dma_start(
                                out=w_c,
                                in_=w_e.ap()[:, ct * P:(ct + 1) * P]
                                    .rearrange("(k p) c -> p k c", p=P)
                                    .bitcast(f32r))
                        load_w.is_load = True
                        yield load_w
                        mm_ps = pp.tile([P, QCS], f32,
                                        name=f"{proj}ps{qc}_{ct}", tag="mm1",
                                        bufs=2)
                        for k in range(FKT):
                            def mm(k=k, mm_ps=mm_ps, w_c=w_c, xck=xc[k]):
                                nc.tensor.matmul(mm_ps[:, :], w_c[:, k, :],
                                                 xck[:, :], start=(k == 0),
                                                 stop=(k == FKT - 1))
                            yield mm
                        if proj == "q":
                            # two zero-padded copies so score matmuls can use
                            # the full 128-row kT-pair stationary (zero rows
                            # contribute nothing to the contraction)
                            qt_e = sb.tile([P, QCS], bf16,
                                           name=f"qte{qc}_{ct}", tag="qte",
                                           bufs=8)
                            qt_o = sb.tile([P, QCS], bf16,
                                           name=f"qto{qc}_{ct}", tag="qto",
                                           bufs=8)
                            qt_all[qc, ct] = (qt_e, qt_o)

                            def cp(qt_e=qt_e, qt_o=qt_o, mm_ps=mm_ps):
                                nc.gpsimd.memset(qt_e[64:128, :], 0.0)
                                nc.gpsimd.memset(qt_o[0:64, :], 0.0)
                                nc.vector.tensor_copy(out=qt_e[0:64, :],
                                                      in_=mm_ps[0:64, :])
                                nc.vector.tensor_copy(out=qt_o[64:128, :],
                                                      in_=mm_ps[64:128, :])
                            yield cp
                        else:
                            def cp(ct=ct, mm_ps=mm_ps):
                                nc.vector.tensor_copy(
                                    out=kT[ct][:, qc * QCS:(qc + 1) * QCS],
                                    in_=mm_ps)
                            yield cp
                if qc == 0:
                    def load_wv0():
                        for k in range(FKT):
                            nc.sync.dma_start(
                                out=wv_t[k],
                                in_=wv_e.ap()[k * P:(k + 1) * P, :]
                                    .bitcast(f32r))
                        for a in range(4):
                            for o in range(2):
                                nc.scalar.dma_start(
                                    out=wp_t[a, o],
                                    in_=wp_e.ap()[a * P:(a + 1) * P,
                                                  o * 512:(o + 1) * 512]
                                        .bitcast(f32r))
                    load_wv0.is_load = True
                    yield load_wv0
                for vt in range(4):
                    v_ps = pp.tile([P, ACH], f32, name=f"vps{qc}_{vt}",
                                   tag="mm1", bufs=2)
                    for k in range(FKT):
                        def mm(k=k, v_ps=v_ps, xck=xc[k], vt=vt):
                            nc.tensor.matmul(v_ps[:, :],
                                             xck[:, vt * P:(vt + 1) * P],
                                             wv_t[k][:, :], start=(k == 0),
                                             stop=(k == FKT - 1))
                        yield mm

                    def vcp(qc=qc, vt=vt, v_ps=v_ps):
                        vxt = vx[qc * 4 + vt]
                        v3 = vxt.rearrange("p (h w) -> p h w", w=P)
                        nc.gpsimd.memset(v3[:, :, HD:HD + 1], 1.0)
                        nc.gpsimd.memset(v3[:, :, HD + 1:P], 0.0)
                        nc.vector.tensor_copy(
                            out=v3[:, :, 0:HD],
                            in_=v_ps.rearrange("p (h d) -> p h d", d=HD))
                    yield vcp

            def cproj_half(qc, ph):
                """Closures for half of c_proj(qc): heads-pair block ph.
                ph=0 accumulates a=0,1 into SBUF f32; ph=1 adds a=2,3 and
                writes the bf16 part tile + DMA."""
                at_tiles = [at_all[qc, j] for j in range(4)]
                for tt in range(4):
                    for oc in range(2):
                        po = pp.tile([P, 512], f32,
                                     name=f"po{qc}_{ph}_{tt}_{oc}",
                                     tag="mm1", bufs=2)
                        for a in (2 * ph, 2 * ph + 1):
                            def mm(a=a, po=po, tt=tt, oc=oc):
                                nc.tensor.matmul(
                                    po[:, :],
                                    at_tiles[a][:, tt * P:(tt + 1) * P],
                                    wp_t[a, oc][:, :],
                                    start=(a % 2 == 0), stop=(a % 2 == 1))
                            yield mm
                        if ph == 0:
                            oa = sb.tile([P, 512], f32,
                                         name=f"oa{qc}_{tt}_{oc}",
                                         tag="oacc", bufs=8)
                            oacc_all[qc, tt, oc] = oa

                            def st_(oa=oa, po=po):
                                nc.vector.tensor_copy(out=oa, in_=po)
                            yield st_
                        else:
                            def st_(qc=qc, tt=tt, oc=oc, po=po):
                                pst = sb.tile([P, 512], bf16,
                                              name=f"pst{qc}_{tt}_{oc}",
                                              tag="pst", bufs=4)
                                nc.vector.tensor_tensor(
                                    out=pst, in0=po,
                                    in1=oacc_all[qc, tt, oc],
                                    op=mybir.AluOpType.add)
                                nc.gpsimd.dma_start(
                                    out=parts[qc][tt * P:(tt + 1) * P,
                                                  oc * 512:(oc + 1) * 512],
                                    in_=pst)
                            yield st_

            def emit_rs(qc):
                rs_insts[qc] = nc.gpsimd.collective_compute(
                    "ReduceScatter", mybir.AluOpType.add,
                    ins=[parts[qc].opt()],
                    outs=[rsos[qc].opt()],
                    replica_groups=rg)

            def emit_attention(qc, fillers):
                """Emit attention for qc, interleaving filler closures
                evenly across the pipeline steps. c_proj(qc) halves are
                injected into the filler stream at the hp=2 boundary (a=0,1
                ready) and right after the loop (a=2,3)."""
                nkt = 4 * qc + 4
                steps_total = 4 * (nkt + SKEW)
                rate = max(0.9, 1.15 * (len(fillers) + 26) / steps_total)
                fi = 0
                budget = 0.0
                at_tiles = [sb.tile([P, QCS], f32r, name=f"at{qc}_{j}",
                                    tag="at", bufs=6) for j in range(4)]
                for j in range(4):
                    at_all[qc, j] = at_tiles[j]
                cp0 = []
                for hp in range(4):
                    if hp == 2:
                        cp0 = list(cproj_half(qc, 0))
                    h_e, h_o = 2 * hp, 2 * hp + 1
                    acc = {}
                    for h, half in ((h_e, 0), (h_o, 64)):
                        acc[h] = pp.tile([P, QCS], f32, name=f"acc{qc}_{h}",
                                         tag="acc", bufs=2)
                    pts = {}
                    for step in range(nkt + SKEW):
                        if step < nkt:
                            kt = step
                            off = (kt - 4 * qc) * P if kt >= 4 * qc else 0
                            # both heads' score tiles share one 2-bank PSUM
                            # tile; exp covers the causally-valid range
                            st = pp.tile([P, 2 * QCS], f32,
                                         name=f"st{qc}_{hp}_{kt}",
                                         tag="st", bufs=2)
                            for h, half in ((h_e, 0), (h_o, 64)):
                                nc.tensor.matmul(
                                    st[:, half * 8 + off:half * 8 + QCS],
                                    kT[hp][:, kt * P:(kt + 1) * P],
                                    qt_all[qc, hp][half // 64][:, off:QCS],
                                    start=True, stop=True)
                            pt = sb.tile([P, 2 * QCS], bf16,
                                         name=f"pt{qc}_{hp}_{kt}",
                                         tag="pt", bufs=4)
                            if off == 0:
                                nc.scalar.activation(out=pt, in_=st,
                                                     func=Exp, scale=0.125)
                            else:
                                for half in (0, 64):
                                    lo = half * 8 + off
                                    hi = half * 8 + QCS
                                    nc.scalar.activation(
                                        out=pt[:, lo:hi], in_=st[:, lo:hi],
                                        func=Exp, scale=0.125)
                            if kt >= 4 * qc:
                                for half in (0, 64):
                                    lo = half * 8 + off
                                    nc.gpsimd.affine_select(
                                        out=pt[:, lo:lo + P],
                                        in_=pt[:, lo:lo + P],
                                        compare_op=mybir.AluOpType.is_ge,
                                        fill=0.0, base=0,
                                        pattern=[[1, P]],
                                        channel_multiplier=-1)
                            pts[kt] = pt
                        if step >= SKEW:
                            kt2 = step - SKEW
                            off2 = (kt2 - 4 * qc) * P if kt2 >= 4 * qc else 0
                            pt2 = pts.pop(kt2)
                            for h, half in ((h_e, 0), (h_o, 64)):
                                nc.tensor.matmul(
                                    acc[h][:, off2:QCS],
                                    vx[kt2][:, h * P:(h + 1) * P],
                                    pt2[:, half * 8 + off2:half * 8 + QCS],
                                    start=(kt2 == 0),
                                    stop=(kt2 == nkt - 1))
                        budget += rate
                        while budget >= 1.0 and (cp0 or fi < len(fillers)):
                            if cp0:
                                cp0.pop(0)()
                            else:
                                fillers[fi]()
                                fi += 1
                            budget -= 1.0
                    for h, half in ((h_e, 0), (h_o, 64)):
                        rsum = sb.tile([1, QCS], f32, name=f"rsum{qc}_{h}",
                                       tag="rs", bufs=2)
                        nc.vector.tensor_copy(out=rsum, in_=acc[h][64:65, :])
                        rs_t = sb.tile([1, QCS], f32, name=f"rst{qc}_{h}",
                                       tag="rs2", bufs=2)
                        nc.vector.reciprocal_approx_fast(out=rs_t, in_=rsum)
                        rb_t = sb.tile([64, QCS], f32, name=f"rb{qc}_{h}",
                                       tag="rb", bufs=2)
                        nc.gpsimd.partition_broadcast(rb_t[:, :], rs_t[:, :])
                        nc.vector.tensor_tensor(
                            out=at_tiles[hp][half:half + 64, :],
                            in0=acc[h][0:64, :], in1=rb_t[:, :],
                            op=mybir.AluOpType.mult)
                # tail-critical second half of c_proj(qc) and its RS first,
                # so the collective trigger isn't queued behind leftover
                # filler work; leftovers then overlap the RS transfer.
                # cp0 leftovers MUST fully drain before half 1 reads oacc.
                while cp0:
                    cp0.pop(0)()
                for u in cproj_half(qc, 1):
                    u()
                emit_rs(qc)
                while fi < len(fillers):
                    fillers[fi]()
                    fi += 1

            # PE warmup: ~10us of dummy matmuls so the HAM clock gate is
            # released before the first real GEMM phase
            wrm = sb.tile([P, QCS], f32r, name="wrm", tag="wrm", bufs=1)
            nc.gpsimd.memset(wrm.bitcast(f32), 0.0)
            for w in range(16):
                wps = pp.tile([P, QCS], f32, name=f"wps{w}", tag="mm1",
                              bufs=2)
                nc.tensor.matmul(wps[:, :], wrm[:, 0:128], wrm[:, :],
                                 start=True, stop=True)

            # dummy tiny ReduceScatter issued up-front: absorbs the cold
            # ncfw/collective staging latency (~tens of us on the first
            # collective) while qkv(0)/att(0) compute, so the real
            # per-chunk RSs run on a warm path
            dums = sb.tile([P, 16], bf16, name="dums", tag="dums", bufs=1)
            dumi = dr.tile([P, 16], bf16, name="dumi", tag="dumi")
            dumo = dr.tile([P // 2, 16], bf16, name="dumo", tag="dumo")
            nc.gpsimd.memset(dums, 0.0)
            nc.gpsimd.dma_start(out=dumi, in_=dums)
            nc.gpsimd.collective_compute(
                "ReduceScatter", mybir.AluOpType.add,
                ins=[dumi.opt()], outs=[dumo.opt()], replica_groups=rg)

            # qkv(0) standalone, then attention(qc) interleaved with
            # qkv(qc+1); c_proj(qc) runs inside its own attention phase.
            # All out-DMAs are pinned at the queue tail after the last RS
            # trigger: an RS-completion wait anywhere mid-stream stalls
            # that engine's whole queue (RS end-to-end latency on the CC
            # core is tens of us). The output is bf16 now, so the tail
            # DMA traffic is only ~2MB.
            for u in qkv_units(0):
                u()
            for qc in range(QCN):
                units = list(qkv_units(qc + 1)) if qc < QCN - 1 else []
                # hoist the first few DMA-load triggers of qkv(qc+1) ahead
                # of att(qc)'s scalar-queue EXPs so the weight gathers
                # prefetch during attention instead of stalling the next
                # qkv phase at the boundary
                pre, rest, nh = [], [], 0
                for u in units:
                    if getattr(u, "is_load", False) and nh < 9:
                        pre.append(u)
                        nh += 1
                    else:
                        rest.append(u)
                for u in pre:
                    u()
                emit_attention(qc, rest)
            # stripe each chunk's final out copy across all four
            # trigger engines' DMA-queue groups; every engine queue is
            # drained by now so RS-completion waits cannot stall work
            engs = [nc.gpsimd, nc.sync, nc.scalar, nc.gpsimd]
            for qc in range(QCN):
                for s in range(4):
                    di = engs[s].dma_start(
                        out=out_e.ap()[qc * 256 + s * 64:
                                       qc * 256 + (s + 1) * 64, :],
                        in_=rsos[qc][s * 64:(s + 1) * 64, :])
                    # pin to the chunk's own RS: chunks 0-2 copies then
                    # fire immediately and overlap the RS(3) mesh
                    add_dep_helper(di.ins, rs_insts[qc].ins,
                                   sync=False,
                                   reason="keep final out DMAs at tail")
    nc.compile()
    return nc


def _get_nc():
    if "nc" not in _CACHE:
        _CACHE["nc"] = _build()
    return _CACHE["nc"]


def _in_maps(x, c_attn_w, c_proj_w):
    maps = []
    for c in range(NCORES):
        b, g = c // 2, c % 2
        h0 = g * HPC
        cols = slice(h0 * HD, h0 * HD + ACH)
        maps.append({
            "xt": np.ascontiguousarray(x[b].T),
            "wq": np.ascontiguousarray(c_attn_w[:, :D][:, cols]),
            "wk": np.ascontiguousarray(c_attn_w[:, D:2 * D][:, cols]),
            "wv": np.ascontiguousarray(c_attn_w[:, 2 * D:][:, cols]),
            "wp": np.ascontiguousarray(c_proj_w[h0 * HD:h0 * HD + ACH, :]),
        })
    return maps


def _run(inputs, trace=False, trace_cores=None):
    from concourse.bass_utils import run_bass_kernel_spmd
    x = np.asarray(inputs["x"], np.float32)
    c_attn_w = np.asarray(inputs["c_attn_w"], np.float32)
    c_attn_b = np.asarray(inputs["c_attn_b"], np.float32)
    c_proj_w = np.asarray(inputs["c_proj_w"], np.float32)
    c_proj_b = np.asarray(inputs["c_proj_b"], np.float32)
    assert not np.any(c_attn_b), "nonzero c_attn_b not supported"

    nc = _get_nc()
    kw = {}
    if trace_cores is not None:
        kw["trace_cores"] = trace_cores
    res = run_bass_kernel_spmd(nc, _in_maps(x, c_attn_w, c_proj_w),
                               core_ids=list(range(NCORES)), trace=trace,
                               **kw)
    out = np.empty((B, S, D), np.float32)
    for c in range(NCORES):
        b, g = c // 2, c % 2
        o = np.asarray(res.results[c]["outp"]).astype(np.float32)
        for qc in range(QCN):
            tok = qc * QCS + g * 256
            out[b, tok:tok + 256, :] = o[qc * 256:(qc + 1) * 256]
    if np.any(c_proj_b):
        out += c_proj_b
    return out, res


def kernel(**inputs):
    out, _ = _run(inputs, trace=False)
    return out


# revision 36
# speedup vs baseline: 1.0314x; 1.0314x over previous
"""TRN2 Bass kernel for GPT-2 style causal self-attention (B=4, S=2048, D=1024, H=16).

Sharding: 8 cores = 4 batches x 2 head-groups (8 heads each).
Each core computes qkv projections for its (batch, head-group), runs causal
attention for its 8 heads, computes a partial c_proj, then a pairwise
ReduceScatter (replica groups [[0,1],[2,3],[4,5],[6,7]]) sums the two
head-group partials and splits the token rows between the pair.

qkv/c_proj matmuls run in float32r (full-rate at 512-wide moving operand);
the attention inner loop (scores, AV) runs in bf16 — 64-contraction and
65-partition-out f32r matmuls measure ~1.5-2x slower than bf16 on HW.
Softmax needs no max-subtraction (scores bounded ~|2.7| at this scale);
masked entries are zeroed after exp via affine_select on a 128-wide window
of the diagonal tile; the softmax denominator rides along as a 65th ones
column of V in the same AV matmul. Scores/exp/AV are causally trimmed to
the valid q-range per k-tile. c_proj for a chunk runs in two half
accumulations (heads 0-3 into SBUF after hp1, heads 4-7 added after hp3)
so the last chunk's c_proj + ReduceScatter trigger right behind the last
AV. External output is bf16 (host upcasts during gather).
"""
import sys
sys.path.insert(0, "/opt/trn_rl_repo")
import numpy as np

B, S, D, H, HD = 4, 2048, 1024, 16, 64
NCORES = 8
HPC = H // 2          # 8 heads per core
ACH = HPC * HD        # 512 local a-channels
P = 128
QCN = 4               # token chunks
QCS = S // QCN        # 512
FKT = D // P          # 8 feature k-tiles
VW = HPC * P          # 1024: per-head [64 v-dims | ones | 63 zeros]
SKEW = 2              # attention pipeline skew (score tiles ahead of AV)

_CACHE = {}


def _build():
    from concourse import bacc, tile, mybir
    from concourse.tile import add_dep_helper
    f32 = mybir.dt.float32
    f32r = mybir.dt.float32r
    bf16 = mybir.dt.bfloat16
    Exp = mybir.ActivationFunctionType.Exp

    nc = bacc.Bacc("TRN2", target_bir_lowering=False, debug=False,
                   num_devices=NCORES)
    xt_e = nc.dram_tensor("xt", [D, S], f32, kind="ExternalInput")
    wq_e = nc.dram_tensor("wq", [D, ACH], f32, kind="ExternalInput")
    wk_e = nc.dram_tensor("wk", [D, ACH], f32, kind="ExternalInput")
    wv_e = nc.dram_tensor("wv", [D, ACH], f32, kind="ExternalInput")
    wp_e = nc.dram_tensor("wp", [ACH, D], f32, kind="ExternalInput")
    out_e = nc.dram_tensor("outp", [S // 2, D], bf16,
                           kind="ExternalOutput")
    rg = [[0, 1], [2, 3], [4, 5], [6, 7]]

    with tile.TileContext(nc) as tc:
        with tc.tile_pool(name="sb", bufs=1) as sb, \
             tc.tile_pool(name="pp", bufs=1, space="PSUM") as pp, \
             tc.tile_pool(name="dr", bufs=1, space="DRAM") as dr:

            kT = [sb.tile([P, S], bf16, name=f"kTr{i}", tag="kT", bufs=4)
                  for i in range(4)]
            vx = [sb.tile([P, VW], bf16, name=f"vxr{i}", tag="vx", bufs=16)
                  for i in range(16)]
            wv_t = [sb.tile([P, ACH], f32r, name=f"wvr{i}", tag="wv", bufs=8)
                    for i in range(FKT)]
            wp_t = {(a, o): sb.tile([P, 512], f32r, name=f"wpr{a}_{o}",
                                    tag="wp", bufs=8)
                    for a in range(4) for o in range(2)}
            parts = [dr.tile([QCS, D], bf16, name=f"part{q}",
                             tag=f"pq{q}") for q in range(QCN)]
            rsos = [dr.tile([QCS // 2, D], bf16, name=f"rso{q}",
                            tag=f"rq{q}") for q in range(QCN)]

            qt_all = {}    # (qc, ct) -> tile
            at_all = {}    # (qc, j) -> tile
            oacc_all = {}  # (qc, tt, oc) -> sbuf f32 accumulator
            rs_insts = {}

            def qkv_units(qc):
                """Generator of emission closures for the qkv phase of qc."""
                xc = [sb.tile([P, QCS], f32r, name=f"xc{qc}_{k}", tag="xc",
                              bufs=10) for k in range(FKT)]

                def load_x():
                    for k in range(FKT):
                        nc.sync.dma_start(
                            out=xc[k],
                            in_=xt_e.ap()[k * P:(k + 1) * P,
                                          qc * QCS:(qc + 1) * QCS]
                                .bitcast(f32r))
                load_x.is_load = True
                yield load_x
                for proj, w_e in (("q", wq_e), ("k", wk_e)):
                    for ct in range(4):
                        w_c = sb.tile([P, FKT, P], f32r,
                                      name=f"w{proj}c{qc}_{ct}", tag="wcol",
                                      bufs=5)

                        def load_w(w_c=w_c, w_e=w_e, ct=ct):
                            nc.scalar.dma_start(
                                out=w_c,
                                in_=w_e.ap()[:, ct * P:(ct + 1) * P]
                                    .rearrange("(k p) c -> p k c", p=P)
                                    .bitcast(f32r))
                        load_w.is_load = True
                        yield load_w
                        mm_ps = pp.tile([P, QCS], f32,
                                        name=f"{proj}ps{qc}_{ct}", tag="mm1",
                                        bufs=2)
                        for k in range(FKT):
                            def mm(k=k, mm_ps=mm_ps, w_c=w_c, xck=xc[k]):
                                nc.tensor.matmul(mm_ps[:, :], w_c[:, k, :],
                                                 xck[:, :], start=(k == 0),
                                                 stop=(k == FKT - 1))
                            yield mm
                        if proj == "q":
                            # two zero-padded copies so score matmuls can use
                            # the full 128-row kT-pair stationary (zero rows
                            # contribute nothing to the contraction)
                            qt_e = sb.tile([P, QCS], bf16,
                                           name=f"qte{qc}_{ct}", tag="qte",
                                           bufs=8)
                            qt_o = sb.tile([P, QCS], bf16,
                                           name=f"qto{qc}_{ct}", tag="qto",
                                           bufs=8)
                            qt_all[qc, ct] = (qt_e, qt_o)

                            def cp(qt_e=qt_e, qt_o=qt_o, mm_ps=mm_ps):
                                nc.gpsimd.memset(qt_e[64:128, :], 0.0)
                                nc.gpsimd.memset(qt_o[0:64, :], 0.0)
                                nc.vector.tensor_copy(out=qt_e[0:64, :],
                                                      in_=mm_ps[0:64, :])
                                nc.vector.tensor_copy(out=qt_o[64:128, :],
                                                      in_=mm_ps[64:128, :])
                            yield cp
                        else:
                            def cp(ct=ct, mm_ps=mm_ps):
                                nc.vector.tensor_copy(
                                    out=kT[ct][:, qc * QCS:(qc + 1) * QCS],
                                    in_=mm_ps)
                            yield cp
                if qc == 0:
                    def load_wv0():
                        for k in range(FKT):
                            nc.sync.dma_start(
                                out=wv_t[k],
                                in_=wv_e.ap()[k * P:(k + 1) * P, :]
                                    .bitcast(f32r))
                        for a in range(4):
                            for o in range(2):
                                nc.scalar.dma_start(
                                    out=wp_t[a, o],
                                    in_=wp_e.ap()[a * P:(a + 1) * P,
                                                  o * 512:(o + 1) * 512]
                                        .bitcast(f32r))
                    load_wv0.is_load = True
                    yield load_wv0
                for vt in range(4):
                    v_ps = pp.tile([P, ACH], f32, name=f"vps{qc}_{vt}",
                                   tag="mm1", bufs=2)
                    for k in range(FKT):
                        def mm(k=k, v_ps=v_ps, xck=xc[k], vt=vt):
                            nc.tensor.matmul(v_ps[:, :],
                                             xck[:, vt * P:(vt + 1) * P],
                                             wv_t[k][:, :], start=(k == 0),
                                             stop=(k == FKT - 1))
                        yield mm

                    def vcp(qc=qc, vt=vt, v_ps=v_ps):
                        vxt = vx[qc * 4 + vt]
                        v3 = vxt.rearrange("p (h w) -> p h w", w=P)
                        nc.gpsimd.memset(v3[:, :, HD:HD + 1], 1.0)
                        nc.gpsimd.memset(v3[:, :, HD + 1:P], 0.0)
                        nc.vector.tensor_copy(
                            out=v3[:, :, 0:HD],
                            in_=v_ps.rearrange("p (h d) -> p h d", d=HD))
                    yield vcp

            def cproj_half(qc, ph):
                """Closures for half of c_proj(qc): heads-pair block ph.
                ph=0 accumulates a=0,1 into SBUF f32; ph=1 adds a=2,3 and
                writes the bf16 part tile + DMA."""
                at_tiles = [at_all[qc, j] for j in range(4)]
                for tt in range(4):
                    for oc in range(2):
                        po = pp.tile([P, 512], f32,
                                     name=f"po{qc}_{ph}_{tt}_{oc}",
                                     tag="mm1", bufs=2)
                        for a in (2 * ph, 2 * ph + 1):
                            def mm(a=a, po=po, tt=tt, oc=oc):
                                nc.tensor.matmul(
                                    po[:, :],
                                    at_tiles[a][:, tt * P:(tt + 1) * P],
                                    wp_t[a, oc][:, :],
                                    start=(a % 2 == 0), stop=(a % 2 == 1))
                            yield mm
                        if ph == 0:
                            oa = sb.tile([P, 512], f32,
                                         name=f"oa{qc}_{tt}_{oc}",
                                         tag="oacc", bufs=8)
                            oacc_all[qc, tt, oc] = oa

                            def st_(oa=oa, po=po):
                                nc.vector.tensor_copy(out=oa, in_=po)
                            yield st_
                        else:
                            def st_(qc=qc, tt=tt, oc=oc, po=po):
                                pst = sb.tile([P, 512], bf16,
                                              name=f"pst{qc}_{tt}_{oc}",
                                              tag="pst", bufs=4)
                                nc.vector.tensor_tensor(
                                    out=pst, in0=po,
                                    in1=oacc_all[qc, tt, oc],
                                    op=mybir.AluOpType.add)
                                nc.gpsimd.dma_start(
                                    out=parts[qc][tt * P:(tt + 1) * P,
                                                  oc * 512:(oc + 1) * 512],
                                    in_=pst)
                            yield st_

            def emit_rs(qc):
                rs_insts[qc] = nc.gpsimd.collective_compute(
                    "ReduceScatter", mybir.AluOpType.add,
                    ins=[parts[qc].opt()],
                    outs=[rsos[qc].opt()],
                    replica_groups=rg)

            def emit_attention(qc, fillers):
                """Emit attention for qc, interleaving filler closures
                evenly across the pipeline steps. c_proj(qc) halves are
                injected into the filler stream at the hp=2 boundary (a=0,1
                ready) and right after the loop (a=2,3)."""
                nkt = 4 * qc + 4
                steps_total = 4 * (nkt + SKEW)
                rate = max(0.9, 1.15 * (len(fillers) + 26) / steps_total)
                fi = 0
                budget = 0.0
                at_tiles = [sb.tile([P, QCS], f32r, name=f"at{qc}_{j}",
                                    tag="at", bufs=6) for j in range(4)]
                for j in range(4):
                    at_all[qc, j] = at_tiles[j]
                cp0 = []
                for hp in range(4):
                    if hp == 2:
                        cp0 = list(cproj_half(qc, 0))
                    h_e, h_o = 2 * hp, 2 * hp + 1
                    acc = {}
                    for h, half in ((h_e, 0), (h_o, 64)):
                        acc[h] = pp.tile([P, QCS], f32, name=f"acc{qc}_{h}",
                                         tag="acc", bufs=2)
                    pts = {}
                    for step in range(nkt + SKEW):
                        if step < nkt:
                            kt = step
                            off = (kt - 4 * qc) * P if kt >= 4 * qc else 0
                            # both heads' score tiles share one 2-bank PSUM
                            # tile; exp covers the causally-valid range
                            st = pp.tile([P, 2 * QCS], f32,
                                         name=f"st{qc}_{hp}_{kt}",
                                         tag="st", bufs=2)
                            for h, half in ((h_e, 0), (h_o, 64)):
                                nc.tensor.matmul(
                                    st[:, half * 8 + off:half * 8 + QCS],
                                    kT[hp][:, kt * P:(kt + 1) * P],
                                    qt_all[qc, hp][half // 64][:, off:QCS],
                                    start=True, stop=True)
                            pt = sb.tile([P, 2 * QCS], bf16,
                                         name=f"pt{qc}_{hp}_{kt}",
                                         tag="pt", bufs=4)
                            if off == 0:
                                nc.scalar.activation(out=pt, in_=st,
                                                     func=Exp, scale=0.125)
                            else:
                                for half in (0, 64):
                                    lo = half * 8 + off
                                    hi = half * 8 + QCS
                                    nc.scalar.activation(
                                        out=pt[:, lo:hi], in_=st[:, lo:hi],
                                        func=Exp, scale=0.125)
                            if kt >= 4 * qc:
                                for half in (0, 64):
                                    lo = half * 8 + off
                                    nc.gpsimd.affine_select(
                                        out=pt[:, lo:lo + P],
                                        in_=pt[:, lo:lo + P],
                                        compare_op=mybir.AluOpType.is_ge,
                                        fill=0.0, base=0,
                                        pattern=[[1, P]],
                                        channel_multiplier=-1)
                            pts[kt] = pt
                        if step >= SKEW:
                            kt2 = step - SKEW
                            off2 = (kt2 - 4 * qc) * P if kt2 >= 4 * qc else 0
                            pt2 = pts.pop(kt2)
                            for h, half in ((h_e, 0), (h_o, 64)):
                                nc.tensor.matmul(
                                    acc[h][:, off2:QCS],
                                    vx[kt2][:, h * P:(h + 1) * P],
                                    pt2[:, half * 8 + off2:half * 8 + QCS],
                                    start=(kt2 == 0),
                                    stop=(kt2 == nkt - 1))
                        budget += rate
                        while budget >= 1.0 and (cp0 or fi < len(fillers)):
                            if cp0:
                                cp0.pop(0)()
                            else:
                                fillers[fi]()
                                fi += 1
                            budget -= 1.0
                    for h, half in ((h_e, 0), (h_o, 64)):
                        rsum = sb.tile([1, QCS], f32, name=f"rsum{qc}_{h}",
                                       tag="rs", bufs=2)
                        nc.vector.tensor_copy(out=rsum, in_=acc[h][64:65, :])
                        rs_t = sb.tile([1, QCS], f32, name=f"rst{qc}_{h}",
                                       tag="rs2", bufs=2)
                        nc.vector.reciprocal_approx_fast(out=rs_t, in_=rsum)
                        rb_t = sb.tile([64, QCS], f32, name=f"rb{qc}_{h}",
                                       tag="rb", bufs=2)
                        nc.gpsimd.partition_broadcast(rb_t[:, :], rs_t[:, :])
                        nc.vector.tensor_tensor(
                            out=at_tiles[hp][half:half + 64, :],
                            in0=acc[h][0:64, :], in1=rb_t[:, :],
                            op=mybir.AluOpType.mult)
                # tail-critical second half of c_proj(qc) and its RS first,
                # so the collective trigger isn't queued behind leftover
                # filler work; leftovers then overlap the RS transfer.
                # cp0 leftovers MUST fully drain before half 1 reads oacc.
                while cp0:
                    cp0.pop(0)()
                for u in cproj_half(qc, 1):
                    u()
                emit_rs(qc)
                while fi < len(fillers):
                    fillers[fi]()
                    fi += 1

            # PE warmup: ~10us of dummy matmuls so the HAM clock gate is
            # released before the first real GEMM phase
            wrm = sb.tile([P, QCS], f32r, name="wrm", tag="wrm", bufs=1)
            nc.gpsimd.memset(wrm.bitcast(f32), 0.0)
            for w in range(16):
                wps = pp.tile([P, QCS], f32, name=f"wps{w}", tag="mm1",
                              bufs=2)
                nc.tensor.matmul(wps[:, :], wrm[:, 0:128], wrm[:, :],
                                 start=True, stop=True)

            # dummy tiny ReduceScatter issued up-front: absorbs the cold
            # ncfw/collective staging latency (~tens of us on the first
            # collective) while qkv(0)/att(0) compute, so the real
            # per-chunk RSs run on a warm path
            dums = sb.tile([P, 16], bf16, name="dums", tag="dums", bufs=1)
            dumi = dr.tile([P, 16], bf16, name="dumi", tag="dumi")
            dumo = dr.tile([P // 2, 16], bf16, name="dumo", tag="dumo")
            nc.gpsimd.memset(dums, 0.0)
            nc.gpsimd.dma_start(out=dumi, in_=dums)
            nc.gpsimd.collective_compute(
                "ReduceScatter", mybir.AluOpType.add,
                ins=[dumi.opt()], outs=[dumo.opt()], replica_groups=rg)

            # qkv(0) standalone, then attention(qc) interleaved with
            # qkv(qc+1); c_proj(qc) runs inside its own attention phase.
            # All out-DMAs are pinned at the queue tail after the last RS
            # trigger: an RS-completion wait anywhere mid-stream stalls
            # that engine's whole queue (RS end-to-end latency on the CC
            # core is tens of us). The output is bf16 now, so the tail
            # DMA traffic is only ~2MB.
            for u in qkv_units(0):
                u()
            for qc in range(QCN):
                units = list(qkv_units(qc + 1)) if qc < QCN - 1 else []
                # hoist the first few DMA-load triggers of qkv(qc+1) ahead
                # of att(qc)'s scalar-queue EXPs so the weight gathers
                # prefetch during attention instead of stalling the next
                # qkv phase at the boundary
                pre, rest, nh = [], [], 0
                for u in units:
                    if getattr(u, "is_load", False) and nh < 5:
                        pre.append(u)
                        nh += 1
                    else:
                        rest.append(u)
                for u in pre:
                    u()
                emit_attention(qc, rest)
            # stripe each chunk's final out copy across all four
            # trigger engines' DMA-queue groups; every engine queue is
            # drained by now so RS-completion waits cannot stall work
            engs = [nc.gpsimd, nc.sync, nc.scalar, nc.gpsimd]
            for qc in range(QCN):
                for s in range(4):
                    di = engs[s].dma_start(
                        out=out_e.ap()[qc * 256 + s * 64:
                                       qc * 256 + (s + 1) * 64, :],
                        in_=rsos[qc][s * 64:(s + 1) * 64, :])
                    add_dep_helper(di.ins, rs_insts[QCN - 1].ins,
                                   sync=False,
                                   reason="keep final out DMAs at tail")
    nc.compile()
    return nc


def _get_nc():
    if "nc" not in _CACHE:
        _CACHE["nc"] = _build()
    return _CACHE["nc"]


def _in_maps(x, c_attn_w, c_proj_w):
    maps = []
    for c in range(NCORES):
        b, g = c // 2, c % 2
        h0 = g * HPC
        cols = slice(h0 * HD, h0 * HD + ACH)
        maps.append({
            "xt": np.ascontiguousarray(x[b].T),
            "wq": np.ascontiguousarray(c_attn_w[:, :D][:, cols]),
            "wk": np.ascontiguousarray(c_attn_w[:, D:2 * D][:, cols]),
            "wv": np.ascontiguousarray(c_attn_w[:, 2 * D:][:, cols]),
            "wp": np.ascontiguousarray(c_proj_w[h0 * HD:h0 * HD + ACH, :]),
        })
    return maps


def _run(inputs, trace=False, trace_cores=None):
    from concourse.bass_utils import run_bass_kernel_spmd
    x = np.asarray(inputs["x"], np.float32)
    c_attn_w = np.asarray(inputs["c_attn_w"], np.float32)
    c_attn_b = np.asarray(inputs["c_attn_b"], np.float32)
    c_proj_w = np.asarray(inputs["c_proj_w"], np.float32)
    c_proj_b = np.asarray(inputs["c_proj_b"], np.float32)
    assert not np.any(c_attn_b), "nonzero c_attn_b not supported"

    nc = _get_nc()
    kw = {}
    if trace_cores is not None:
        kw["trace_cores"] = trace_cores
    res = run_bass_kernel_spmd(nc, _in_maps(x, c_attn_w, c_proj_w),
                               core_ids=list(range(NCORES)), trace=trace,
                               **kw)
    out = np.empty((B, S, D), np.float32)
    for c in range(NCORES):
        b, g = c // 2, c % 2
        o = np.asarray(res.results[c]["outp"]).astype(np.float32)
        for qc in range(QCN):
            tok = qc * QCS + g * 256
            out[b, tok:tok + 256, :] = o[qc * 256:(qc + 1) * 256]
    if np.any(c_proj_b):
        out += c_proj_b
    return out, res


def kernel(**inputs):
    out, _ = _run(inputs, trace=False)
    return out


# revision 38
# speedup vs baseline: 1.0611x; 1.0288x over previous
"""TRN2 Bass kernel for GPT-2 style causal self-attention (B=4, S=2048, D=1024, H=16).

Sharding: 8 cores = 4 batches x 2 head-groups (8 heads each).
Each core computes qkv projections for its (batch, head-group), runs causal
attention for its 8 heads, computes a partial c_proj, then a pairwise
ReduceScatter (replica groups [[0,1],[2,3],[4,5],[6,7]]) sums the two
head-group partials and splits the token rows between the pair.

qkv/c_proj matmuls run in float32r (full-rate at 512-wide moving operand);
the attention inner loop (scores, AV) runs in bf16 — 64-contraction and
65-partition-out f32r matmuls measure ~1.5-2x slower than bf16 on HW.
Softmax needs no max-subtraction (scores bounded ~|2.7| at this scale);
masked entries are zeroed after exp via affine_select on a 128-wide window
of the diagonal tile; the softmax denominator rides along as a 65th ones
column of V in the same AV matmul. Scores/exp/AV are causally trimmed to
the valid q-range per k-tile. c_proj for a chunk runs in two half
accumulations (heads 0-3 into SBUF after hp1, heads 4-7 added after hp3)
so the last chunk's c_proj + ReduceScatter trigger right behind the last
AV. External output is bf16 (host upcasts during gather).
"""
import sys
sys.path.insert(0, "/opt/trn_rl_repo")
import numpy as np

B, S, D, H, HD = 4, 2048, 1024, 16, 64
NCORES = 8
HPC = H // 2          # 8 heads per core
ACH = HPC * HD        # 512 local a-channels
P = 128
QCN = 4               # token chunks
QCS = S // QCN        # 512
FKT = D // P          # 8 feature k-tiles
VW = HPC * P          # 1024: per-head [64 v-dims | ones | 63 zeros]
SKEW = 2              # attention pipeline skew (score tiles ahead of AV)

_CACHE = {}


def _build():
    from concourse import bacc, tile, mybir
    from concourse.tile import add_dep_helper
    f32 = mybir.dt.float32
    f32r = mybir.dt.float32r
    bf16 = mybir.dt.bfloat16
    Exp = mybir.ActivationFunctionType.Exp

    nc = bacc.Bacc("TRN2", target_bir_lowering=False, debug=False,
                   num_devices=NCORES)
    xt_e = nc.dram_tensor("xt", [D, S], f32, kind="ExternalInput")
    wq_e = nc.dram_tensor("wq", [D, ACH], f32, kind="ExternalInput")
    wk_e = nc.dram_tensor("wk", [D, ACH], f32, kind="ExternalInput")
    wv_e = nc.dram_tensor("wv", [D, ACH], f32, kind="ExternalInput")
    wp_e = nc.dram_tensor("wp", [ACH, D], f32, kind="ExternalInput")
    out_e = nc.dram_tensor("outp", [S // 2, D], bf16,
                           kind="ExternalOutput")
    rg = [[0, 1], [2, 3], [4, 5], [6, 7]]

    with tile.TileContext(nc) as tc:
        with tc.tile_pool(name="sb", bufs=1) as sb, \
             tc.tile_pool(name="pp", bufs=1, space="PSUM") as pp, \
             tc.tile_pool(name="dr", bufs=1, space="DRAM") as dr:

            kT = [sb.tile([P, S], bf16, name=f"kTr{i}", tag="kT", bufs=4)
                  for i in range(4)]
            vx = [sb.tile([P, VW], bf16, name=f"vxr{i}", tag="vx", bufs=16)
                  for i in range(16)]
            wv_t = [sb.tile([P, ACH], f32r, name=f"wvr{i}", tag="wv", bufs=8)
                    for i in range(FKT)]
            wp_t = {(a, o): sb.tile([P, 512], f32r, name=f"wpr{a}_{o}",
                                    tag="wp", bufs=8)
                    for a in range(4) for o in range(2)}
            parts = [dr.tile([QCS, D], bf16, name=f"part{q}",
                             tag=f"pq{q}") for q in range(QCN)]
            rsos = [dr.tile([QCS // 2, D], bf16, name=f"rso{q}",
                            tag=f"rq{q}") for q in range(QCN)]

            qt_all = {}    # (qc, ct) -> tile
            at_all = {}    # (qc, j) -> tile
            oacc_all = {}  # (qc, tt, oc) -> sbuf f32 accumulator
            rs_insts = {}

            def qkv_units(qc):
                """Generator of emission closures for the qkv phase of qc."""
                xc = [sb.tile([P, QCS], f32r, name=f"xc{qc}_{k}", tag="xc",
                              bufs=10) for k in range(FKT)]

                def load_x():
                    for k in range(FKT):
                        nc.sync.dma_start(
                            out=xc[k],
                            in_=xt_e.ap()[k * P:(k + 1) * P,
                                          qc * QCS:(qc + 1) * QCS]
                                .bitcast(f32r))
                load_x.is_load = True
                yield load_x
                for proj, w_e in (("q", wq_e), ("k", wk_e)):
                    for ct in range(4):
                        w_c = sb.tile([P, FKT, P], f32r,
                                      name=f"w{proj}c{qc}_{ct}", tag="wcol",
                                      bufs=5)

                        def load_w(w_c=w_c, w_e=w_e, ct=ct):
                            nc.scalar.dma_start(
                                out=w_c,
                                in_=w_e.ap()[:, ct * P:(ct + 1) * P]
                                    .rearrange("(k p) c -> p k c", p=P)
                                    .bitcast(f32r))
                        load_w.is_load = True
                        yield load_w
                        mm_ps = pp.tile([P, QCS], f32,
                                        name=f"{proj}ps{qc}_{ct}", tag="mm1",
                                        bufs=2)
                        for k in range(FKT):
                            def mm(k=k, mm_ps=mm_ps, w_c=w_c, xck=xc[k]):
                                nc.tensor.matmul(mm_ps[:, :], w_c[:, k, :],
                                                 xck[:, :], start=(k == 0),
                                                 stop=(k == FKT - 1))
                            yield mm
                        if proj == "q":
                            # two zero-padded copies so score matmuls can use
                            # the full 128-row kT-pair stationary (zero rows
                            # contribute nothing to the contraction)
                            qt_e = sb.tile([P, QCS], bf16,
                                           name=f"qte{qc}_{ct}", tag="qte",
                                           bufs=8)
                            qt_o = sb.tile([P, QCS], bf16,
                                           name=f"qto{qc}_{ct}", tag="qto",
                                           bufs=8)
                            qt_all[qc, ct] = (qt_e, qt_o)

                            def cp(qt_e=qt_e, qt_o=qt_o, mm_ps=mm_ps):
                                nc.gpsimd.memset(qt_e[64:128, :], 0.0)
                                nc.gpsimd.memset(qt_o[0:64, :], 0.0)
                                nc.vector.tensor_copy(out=qt_e[0:64, :],
                                                      in_=mm_ps[0:64, :])
                                nc.vector.tensor_copy(out=qt_o[64:128, :],
                                                      in_=mm_ps[64:128, :])
                            yield cp
                        else:
                            def cp(ct=ct, mm_ps=mm_ps):
                                nc.vector.tensor_copy(
                                    out=kT[ct][:, qc * QCS:(qc + 1) * QCS],
                                    in_=mm_ps)
                            yield cp
                if qc == 0:
                    def load_wv0():
                        for k in range(FKT):
                            nc.sync.dma_start(
                                out=wv_t[k],
                                in_=wv_e.ap()[k * P:(k + 1) * P, :]
                                    .bitcast(f32r))
                        for a in range(4):
                            for o in range(2):
                                nc.scalar.dma_start(
                                    out=wp_t[a, o],
                                    in_=wp_e.ap()[a * P:(a + 1) * P,
                                                  o * 512:(o + 1) * 512]
                                        .bitcast(f32r))
                    load_wv0.is_load = True
                    yield load_wv0
                for vt in range(4):
                    v_ps = pp.tile([P, ACH], f32, name=f"vps{qc}_{vt}",
                                   tag="mm1", bufs=2)
                    for k in range(FKT):
                        def mm(k=k, v_ps=v_ps, xck=xc[k], vt=vt):
                            nc.tensor.matmul(v_ps[:, :],
                                             xck[:, vt * P:(vt + 1) * P],
                                             wv_t[k][:, :], start=(k == 0),
                                             stop=(k == FKT - 1))
                        yield mm

                    def vcp(qc=qc, vt=vt, v_ps=v_ps):
                        vxt = vx[qc * 4 + vt]
                        v3 = vxt.rearrange("p (h w) -> p h w", w=P)
                        nc.gpsimd.memset(v3[:, :, HD:HD + 1], 1.0)
                        nc.gpsimd.memset(v3[:, :, HD + 1:P], 0.0)
                        nc.vector.tensor_copy(
                            out=v3[:, :, 0:HD],
                            in_=v_ps.rearrange("p (h d) -> p h d", d=HD))
                    yield vcp

            def cproj_half(qc, ph):
                """Closures for half of c_proj(qc): heads-pair block ph.
                ph=0 accumulates a=0,1 into SBUF f32; ph=1 adds a=2,3 and
                writes the bf16 part tile + DMA."""
                at_tiles = [at_all[qc, j] for j in range(4)]
                for tt in range(4):
                    for oc in range(2):
                        po = pp.tile([P, 512], f32,
                                     name=f"po{qc}_{ph}_{tt}_{oc}",
                                     tag="mm1", bufs=2)
                        for a in (2 * ph, 2 * ph + 1):
                            def mm(a=a, po=po, tt=tt, oc=oc):
                                nc.tensor.matmul(
                                    po[:, :],
                                    at_tiles[a][:, tt * P:(tt + 1) * P],
                                    wp_t[a, oc][:, :],
                                    start=(a % 2 == 0), stop=(a % 2 == 1))
                            yield mm
                        if ph == 0:
                            oa = sb.tile([P, 512], f32,
                                         name=f"oa{qc}_{tt}_{oc}",
                                         tag="oacc", bufs=8)
                            oacc_all[qc, tt, oc] = oa

                            def st_(oa=oa, po=po):
                                nc.vector.tensor_copy(out=oa, in_=po)
                            yield st_
                        else:
                            def st_(qc=qc, tt=tt, oc=oc, po=po):
                                pst = sb.tile([P, 512], bf16,
                                              name=f"pst{qc}_{tt}_{oc}",
                                              tag="pst", bufs=4)
                                nc.vector.tensor_tensor(
                                    out=pst, in0=po,
                                    in1=oacc_all[qc, tt, oc],
                                    op=mybir.AluOpType.add)
                                nc.gpsimd.dma_start(
                                    out=parts[qc][tt * P:(tt + 1) * P,
                                                  oc * 512:(oc + 1) * 512],
                                    in_=pst)
                            yield st_

            def emit_rs(qc):
                rs_insts[qc] = nc.gpsimd.collective_compute(
                    "ReduceScatter", mybir.AluOpType.add,
                    ins=[parts[qc].opt()],
                    outs=[rsos[qc].opt()],
                    replica_groups=rg)

            def emit_attention(qc, fillers):
                """Emit attention for qc, interleaving filler closures
                evenly across the pipeline steps. c_proj(qc) halves are
                injected into the filler stream at the hp=2 boundary (a=0,1
                ready) and right after the loop (a=2,3)."""
                nkt = 4 * qc + 4
                steps_total = 4 * (nkt + SKEW)
                rate = max(0.9, 1.15 * (len(fillers) + 26) / steps_total)
                fi = 0
                budget = 0.0
                at_tiles = [sb.tile([P, QCS], f32r, name=f"at{qc}_{j}",
                                    tag="at", bufs=6) for j in range(4)]
                for j in range(4):
                    at_all[qc, j] = at_tiles[j]
                cp0 = []
                for hp in range(4):
                    if hp == 2:
                        cp0 = list(cproj_half(qc, 0))
                    h_e, h_o = 2 * hp, 2 * hp + 1
                    acc = {}
                    for h, half in ((h_e, 0), (h_o, 64)):
                        acc[h] = pp.tile([P, QCS], f32, name=f"acc{qc}_{h}",
                                         tag="acc", bufs=2)
                    pts = {}
                    for step in range(nkt + SKEW):
                        if step < nkt:
                            kt = step
                            off = (kt - 4 * qc) * P if kt >= 4 * qc else 0
                            # both heads' score tiles share one 2-bank PSUM
                            # tile; exp covers the causally-valid range
                            st = pp.tile([P, 2 * QCS], f32,
                                         name=f"st{qc}_{hp}_{kt}",
                                         tag="st", bufs=2)
                            for h, half in ((h_e, 0), (h_o, 64)):
                                nc.tensor.matmul(
                                    st[:, half * 8 + off:half * 8 + QCS],
                                    kT[hp][:, kt * P:(kt + 1) * P],
                                    qt_all[qc, hp][half // 64][:, off:QCS],
                                    start=True, stop=True)
                            pt = sb.tile([P, 2 * QCS], bf16,
                                         name=f"pt{qc}_{hp}_{kt}",
                                         tag="pt", bufs=4)
                            if off == 0:
                                nc.scalar.activation(out=pt, in_=st,
                                                     func=Exp, scale=0.125)
                            else:
                                for half in (0, 64):
                                    lo = half * 8 + off
                                    hi = half * 8 + QCS
                                    nc.scalar.activation(
                                        out=pt[:, lo:hi], in_=st[:, lo:hi],
                                        func=Exp, scale=0.125)
                            if kt >= 4 * qc:
                                for half in (0, 64):
                                    lo = half * 8 + off
                                    nc.gpsimd.affine_select(
                                        out=pt[:, lo:lo + P],
                                        in_=pt[:, lo:lo + P],
                                        compare_op=mybir.AluOpType.is_ge,
                                        fill=0.0, base=0,
                                        pattern=[[1, P]],
                                        channel_multiplier=-1)
                            pts[kt] = pt
                        if step >= SKEW:
                            kt2 = step - SKEW
                            off2 = (kt2 - 4 * qc) * P if kt2 >= 4 * qc else 0
                            pt2 = pts.pop(kt2)
                            for h, half in ((h_e, 0), (h_o, 64)):
                                nc.tensor.matmul(
                                    acc[h][:, off2:QCS],
                                    vx[kt2][:, h * P:(h + 1) * P],
                                    pt2[:, half * 8 + off2:half * 8 + QCS],
                                    start=(kt2 == 0),
                                    stop=(kt2 == nkt - 1))
                        budget += rate
                        while budget >= 1.0 and fi < len(fillers):
                            fillers[fi]()
                            fi += 1
                            budget -= 1.0
                    for h, half in ((h_e, 0), (h_o, 64)):
                        rsum = sb.tile([1, QCS], f32, name=f"rsum{qc}_{h}",
                                       tag="rs", bufs=2)
                        nc.vector.tensor_copy(out=rsum, in_=acc[h][64:65, :])
                        rs_t = sb.tile([1, QCS], f32, name=f"rst{qc}_{h}",
                                       tag="rs2", bufs=2)
                        nc.vector.reciprocal_approx_fast(out=rs_t, in_=rsum)
                        rb_t = sb.tile([64, QCS], f32, name=f"rb{qc}_{h}",
                                       tag="rb", bufs=2)
                        nc.gpsimd.partition_broadcast(rb_t[:, :], rs_t[:, :])
                        nc.vector.tensor_tensor(
                            out=at_tiles[hp][half:half + 64, :],
                            in0=acc[h][0:64, :], in1=rb_t[:, :],
                            op=mybir.AluOpType.mult)
                # tail-critical second half of c_proj(qc) and its RS first,
                # so the collective trigger isn't queued behind leftover
                # filler work; leftovers then overlap the RS transfer.
                # cp0 leftovers MUST fully drain before half 1 reads oacc.
                while cp0:
                    cp0.pop(0)()
                for u in cproj_half(qc, 1):
                    u()
                emit_rs(qc)
                while fi < len(fillers):
                    fillers[fi]()
                    fi += 1

            # PE warmup: ~10us of dummy matmuls so the HAM clock gate is
            # released before the first real GEMM phase
            wrm = sb.tile([P, QCS], f32r, name="wrm", tag="wrm", bufs=1)
            nc.gpsimd.memset(wrm.bitcast(f32), 0.0)
            for w in range(16):
                wps = pp.tile([P, QCS], f32, name=f"wps{w}", tag="mm1",
                              bufs=2)
                nc.tensor.matmul(wps[:, :], wrm[:, 0:128], wrm[:, :],
                                 start=True, stop=True)

            # dummy tiny ReduceScatter issued up-front: absorbs the cold
            # ncfw/collective staging latency (~tens of us on the first
            # collective) while qkv(0)/att(0) compute, so the real
            # per-chunk RSs run on a warm path
            dums = sb.tile([P, 16], bf16, name="dums", tag="dums", bufs=1)
            dumi = dr.tile([P, 16], bf16, name="dumi", tag="dumi")
            dumo = dr.tile([P // 2, 16], bf16, name="dumo", tag="dumo")
            nc.gpsimd.memset(dums, 0.0)
            nc.gpsimd.dma_start(out=dumi, in_=dums)
            nc.gpsimd.collective_compute(
                "ReduceScatter", mybir.AluOpType.add,
                ins=[dumi.opt()], outs=[dumo.opt()], replica_groups=rg)

            # qkv(0) standalone, then attention(qc) interleaved with
            # qkv(qc+1); c_proj(qc) runs inside its own attention phase.
            # All out-DMAs are pinned at the queue tail after the last RS
            # trigger: an RS-completion wait anywhere mid-stream stalls
            # that engine's whole queue (RS end-to-end latency on the CC
            # core is tens of us). The output is bf16 now, so the tail
            # DMA traffic is only ~2MB.
            for u in qkv_units(0):
                u()
            for qc in range(QCN):
                units = list(qkv_units(qc + 1)) if qc < QCN - 1 else []
                # hoist the first few DMA-load triggers of qkv(qc+1) ahead
                # of att(qc)'s scalar-queue EXPs so the weight gathers
                # prefetch during attention instead of stalling the next
                # qkv phase at the boundary
                pre, rest, nh = [], [], 0
                for u in units:
                    if getattr(u, "is_load", False) and nh < 5:
                        pre.append(u)
                        nh += 1
                    else:
                        rest.append(u)
                for u in pre:
                    u()
                emit_attention(qc, rest)
            # stripe each chunk's final out copy across all four
            # trigger engines' DMA-queue groups; every engine queue is
            # drained by now so RS-completion waits cannot stall work
            engs = [nc.gpsimd, nc.sync, nc.scalar, nc.gpsimd]
            for qc in range(QCN):
                for s in range(4):
                    di = engs[s].dma_start(
                        out=out_e.ap()[qc * 256 + s * 64:
                                       qc * 256 + (s + 1) * 64, :],
                        in_=rsos[qc][s * 64:(s + 1) * 64, :])
                    add_dep_helper(di.ins, rs_insts[QCN - 1].ins,
                                   sync=False,
                                   reason="keep final out DMAs at tail")
    nc.compile()
    return nc


def _get_nc():
    if "nc" not in _CACHE:
        _CACHE["nc"] = _build()
    return _CACHE["nc"]


def _in_maps(x, c_attn_w, c_proj_w):
    maps = []
    for c in range(NCORES):
        b, g = c // 2, c % 2
        h0 = g * HPC
        cols = slice(h0 * HD, h0 * HD + ACH)
        maps.append({
            "xt": np.ascontiguousarray(x[b].T),
            "wq": np.ascontiguousarray(c_attn_w[:, :D][:, cols]),
            "wk": np.ascontiguousarray(c_attn_w[:, D:2 * D][:, cols]),
            "wv": np.ascontiguousarray(c_attn_w[:, 2 * D:][:, cols]),
            "wp": np.ascontiguousarray(c_proj_w[h0 * HD:h0 * HD + ACH, :]),
        })
    return maps


def _run(inputs, trace=False, trace_cores=None):
    from concourse.bass_utils import run_bass_kernel_spmd
    x = np.asarray(inputs["x"], np.float32)
    c_attn_w = np.asarray(inputs["c_attn_w"], np.float32)
    c_attn_b = np.asarray(inputs["c_attn_b"], np.float32)
    c_proj_w = np.asarray(inputs["c_proj_w"], np.float32)
    c_proj_b = np.asarray(inputs["c_proj_b"], np.float32)
    assert not np.any(c_attn_b), "nonzero c_attn_b not supported"

    nc = _get_nc()
    kw = {}
    if trace_cores is not None:
        kw["trace_cores"] = trace_cores
    res = run_bass_kernel_spmd(nc, _in_maps(x, c_attn_w, c_proj_w),
                               core_ids=list(range(NCORES)), trace=trace,
                               **kw)
    out = np.empty((B, S, D), np.float32)
    for c in range(NCORES):
        b, g = c // 2, c % 2
        o = np.asarray(res.results[c]["outp"]).astype(np.float32)
        for qc in range(QCN):
            tok = qc * QCS + g * 256
            out[b, tok:tok + 256, :] = o[qc * 256:(qc + 1) * 256]
    if np.any(c_proj_b):
        out += c_proj_b
    return out, res


def kernel(**inputs):
    out, _ = _run(inputs, trace=False)
    return out
